# revision 12
# baseline (speedup 1.0000x reference)
"""TRN2 Bass kernel for nn_MultiHeadAttention_82411832476301.

Full inputs in, full output out. Sharding: 8 cores = 4 batches x 2
head-groups (8 heads each). All matmul operands bf16 (PSUM accumulates
fp32). Design minimizes total engine work and DMA descriptor count
(measured: DMA is ~30-40ns/segment descriptor-bound; cross-engine
concurrency is weak, so evicts go to whichever engine idles in that
phase):

  - All DRAM tensors pre-tiled [128, C*N] so each DMA is one contiguous
    8-32KB descriptor per partition (13 DMAs total).
  - Phase A: Q/K projections as 32-MM psum chains into [128, 2048],
    ScalarE Identity-evict with fused bias (+1/8 scale for Q, bias
    pre-divided on host); V projection into vaug[st] [128, 520] with
    gpsimd-memset ones columns (65th col per head = softmax denominator
    trick); ScalarE strided copy-evict.
  - Phase B per (pair, q-block of 1024): per k-chunk, 4 score matmuls
    into one joint PSUM tile [128, 2048] (h0 cols 0-1023, h1 cols
    1024-2047), ONE exp activation N=2048 (best ScalarE overhead
    amortization), 4 context matmuls accumulating [65, 2048] joint ctx
    (row 64 = denominator). Tail: one fast DVE evict (frees psum),
    bf16 reciprocal + gpsimd partition_broadcast + 2x-mode DVE
    multiplies into ctxP.
  - Phase C: single output projection (4-pair accumulation), ScalarE
    evicts, 4 batched output DMAs, out bf16 in tiled layout.
Host combines: out[b] = untile(core(2b).out) + untile(core(2b+1).out)
+ bo (fp32).
"""

import sys

if "/opt/trn_rl_repo" not in sys.path:
    sys.path.insert(0, "/opt/trn_rl_repo")

import numpy as np
from contextlib import ExitStack

import concourse.bass as bass
import concourse.mybir as mybir
import concourse.tile as tile
from concourse import bacc
from concourse import bass_utils

P = 128
BS = 4
S = 2048          # sequence length
D = 1024          # model dim
NH = 16           # total heads
HD = 64           # head dim
G = 8             # heads per group (per core)
GD = G * HD       # 512 dims per group
GDA = G * (HD + 1)  # 520: v-dims augmented with ones columns
QB = 1024         # q block size
NQB = S // QB     # 2
KT = S // P       # 16 k-chunks of 128
DT = mybir.dt.float32
BF = mybir.dt.bfloat16
FP = mybir.ActivationFunctionType
ALU = mybir.AluOpType


def _emit_kernel(nc):
    # tiled layouts: x_tiled[p, c*N + n] = x[c*128 + p, n] -> per-partition
    # rows are one contiguous DMA descriptor each
    inputT = nc.dram_tensor("inputT", (P, 8 * S), BF, kind="ExternalInput").ap()
    wqT = nc.dram_tensor("wqT", (P, 8 * GD), BF, kind="ExternalInput").ap()
    wkT = nc.dram_tensor("wkT", (P, 8 * GD), BF, kind="ExternalInput").ap()
    wvT = nc.dram_tensor("wvT", (P, 8 * GD), BF, kind="ExternalInput").ap()
    woT = nc.dram_tensor("woT", (P, 4 * D), BF, kind="ExternalInput").ap()
    bq_d = nc.dram_tensor("bq", (P, 4), DT, kind="ExternalInput").ap()
    bk_d = nc.dram_tensor("bk", (P, 4), DT, kind="ExternalInput").ap()
    bv_d = nc.dram_tensor("bv", (1, GD), BF, kind="ExternalInput").ap()
    out_d = nc.dram_tensor("out", (P, 16 * D), BF, kind="ExternalOutput").ap()

    with tile.TileContext(nc) as tc:
        _body(nc, tc, inputT, wqT, wkT, wvT, woT, bq_d, bk_d, bv_d,
              out_d)
    return nc


def _body(nc, tc, inputT, wqT, wkT, wvT, woT, bq_d, bk_d, bv_d,
          out_d):
    with ExitStack() as l0:
        pconst = l0.enter_context(tc.tile_pool(name="const", bufs=1))
        pqkv = l0.enter_context(tc.tile_pool(name="qkv", bufs=1))

        ones_t = pconst.tile([1, P], BF, tag="ones", name="ones_t")
        nc.gpsimd.memset(ones_t[:], 1.0)
        bq_sb = pconst.tile([P, 4], DT, tag="bq", name="bq_sb")
        nc.sync.dma_start(bq_sb[:], bq_d[:])
        bk_sb = pconst.tile([P, 4], DT, tag="bk", name="bk_sb")
        nc.sync.dma_start(bk_sb[:], bk_d[:])
        bv_sb = pconst.tile([1, GD], BF, tag="bv", name="bv_sb")
        nc.sync.dma_start(bv_sb[:], bv_d[:])

        qT = [pqkv.tile([P, S], BF, tag=f"q{ec}", name=f"qT{ec}")
              for ec in range(4)]
        kT = [pqkv.tile([P, S], BF, tag=f"k{ec}", name=f"kT{ec}")
              for ec in range(4)]
        vaug = [pqkv.tile([P, GDA], BF, tag=f"v{st}", name=f"vaug{st}")
                for st in range(KT)]

        # ================= Phase A: projections =================
        with ExitStack() as la:
            pin = la.enter_context(tc.tile_pool(name="pin", bufs=1))
            pw = la.enter_context(tc.tile_pool(name="pw", bufs=1))
            ppv = la.enter_context(
                tc.tile_pool(name="psAV", bufs=2, space="PSUM"))
            ppa = la.enter_context(
                tc.tile_pool(name="psA", bufs=1, space="PSUM"))

            in_big = pin.tile([P, 8 * S], BF, tag="inb", name="in_big")
            nc.sync.dma_start(in_big[:], inputT[:])
            int_t = [in_big[:, dc * S:(dc + 1) * S] for dc in range(8)]
            wv_big = pw.tile([P, 8 * GD], BF, tag="wvb", name="wv_big")
            nc.sync.dma_start(wv_big[:], wvT[:])
            wv_t = [wv_big[:, dc * GD:(dc + 1) * GD] for dc in range(8)]
            wst = {}
            for p, wdram in enumerate((wqT, wkT)):
                big = pw.tile([P, 8 * GD], BF, tag=f"wb{p}", name=f"w_big{p}")
                nc.sync.dma_start(big[:], wdram[:])
                for dc in range(8):
                    wst[p, dc] = big[:, dc * GD:(dc + 1) * GD]

            # V projection (512-wide); ones columns set by gpsimd memset
            for st in range(KT):
                ones_cols = vaug[st][:].rearrange(
                    "p (h c) -> p h c", c=HD + 1)[:, :, HD:HD + 1]
                nc.gpsimd.memset(ones_cols, 1.0)
                ps = ppv.tile([P, GD], DT, tag="psv", name=f"psV{st}")
                for dc in range(8):
                    lhs = int_t[dc][:, st * P:(st + 1) * P]
                    nc.tensor.matmul(ps[:], lhsT=lhs, rhs=wv_t[dc],
                                     start=(dc == 0), stop=False)
                nc.tensor.matmul(ps[:], lhsT=ones_t[0:1, 0:P],
                                 rhs=bv_sb[0:1, :],
                                 start=False, stop=True)
                dst3 = vaug[st][:].rearrange(
                    "p (h c) -> p h c", c=HD + 1)[:, :, 0:HD]
                nc.scalar.copy(dst3, ps[:].rearrange(
                    "p (h c) -> p h c", c=HD))

            # Q/K projections: full-row psum chains, one evict per pair
            for p in range(2):
                for ec in range(4):
                    ps = ppa.tile([P, S], DT, tag="psqk",
                                  name=f"psA{p}_{ec}")
                    for sb in range(4):
                        sl = slice(sb * 512, sb * 512 + 512)
                        for dc in range(8):
                            nc.tensor.matmul(
                                ps[:, sl],
                                lhsT=wst[p, dc][:, ec * P:(ec + 1) * P],
                                rhs=int_t[dc][:, sl],
                                start=(dc == 0), stop=(dc == 7))
                    dest = (qT if p == 0 else kT)[ec][:]
                    bias = (bq_sb if p == 0 else bk_sb)[:, ec:ec + 1]
                    nc.scalar.activation(
                        dest, ps[:], FP.Identity, bias=bias,
                        scale=(0.125 if p == 0 else 1.0))

        # prefetch output-projection weights (DMA idle during attention)
        pwo = l0.enter_context(tc.tile_pool(name="pwo", bufs=1))
        wo_big = pwo.tile([P, 4 * D], BF, tag="wob", name="wo_big")
        nc.sync.dma_start(wo_big[:], woT[:])
        wo_t = [wo_big[:, cc * D:(cc + 1) * D] for cc in range(4)]

        # ================= Phase B: attention =================
        pctx = l0.enter_context(tc.tile_pool(name="ctxp", bufs=1))
        ctxP = [pctx.tile([P, S], BF, tag=f"ctx{cc}", name=f"ctxP{cc}")
                for cc in range(4)]

        lb = ExitStack()
        pet = lb.enter_context(tc.tile_pool(name="et", bufs=2))
        prr = lb.enter_context(tc.tile_pool(name="rr", bufs=2))
        pps = lb.enter_context(tc.tile_pool(name="psS", bufs=1, space="PSUM"))
        ppc = lb.enter_context(tc.tile_pool(name="psC", bufs=1, space="PSUM"))

        for pair in range(4):
            for qb in range(NQB):
                it = pair * NQB + qb
                q0 = qb * QB
                ps_s = pps.tile([P, 2 * QB], DT, tag="pss", name=f"pss{it}")
                ps_c = ppc.tile([HD + 1, 2 * QB], DT, tag="psc",
                                name=f"psc{it}")
                for kt in range(KT):
                    lk0 = kT[pair][0:HD, kt * P:(kt + 1) * P]
                    lk1 = kT[pair][HD:P, kt * P:(kt + 1) * P]
                    first, last = kt == 0, kt == KT - 1
                    # scores: h0 -> cols 0:1024, h1 -> cols 1024:2048
                    nc.tensor.matmul(ps_s[:, 0:512], lhsT=lk0,
                                     rhs=qT[pair][0:HD, q0:q0 + 512],
                                     start=True, stop=True)
                    nc.tensor.matmul(ps_s[:, 512:1024], lhsT=lk0,
                                     rhs=qT[pair][0:HD, q0 + 512:q0 + QB],
                                     start=True, stop=True)
                    nc.tensor.matmul(ps_s[:, 1024:1536], lhsT=lk1,
                                     rhs=qT[pair][HD:P, q0:q0 + 512],
                                     start=True, stop=True)
                    nc.tensor.matmul(ps_s[:, 1536:2048], lhsT=lk1,
                                     rhs=qT[pair][HD:P, q0 + 512:q0 + QB],
                                     start=True, stop=True)
                    et = pet.tile([P, 2 * QB], BF, tag="et",
                                  name=f"et{it}_{kt}")
                    nc.scalar.activation(et[:], ps_s[:], FP.Exp)
                    # context accumulation (65-row: row 64 = denominator)
                    lv0 = vaug[kt][:, (2 * pair) * 65:(2 * pair) * 65 + 65]
                    lv1 = vaug[kt][:, (2 * pair + 1) * 65:
                                   (2 * pair + 1) * 65 + 65]
                    nc.tensor.matmul(ps_c[:, 0:512], lhsT=lv0,
                                     rhs=et[:, 0:512],
                                     start=first, stop=last)
                    nc.tensor.matmul(ps_c[:, 512:1024], lhsT=lv0,
                                     rhs=et[:, 512:1024],
                                     start=first, stop=last)
                    nc.tensor.matmul(ps_c[:, 1024:1536], lhsT=lv1,
                                     rhs=et[:, 1024:1536],
                                     start=first, stop=last)
                    nc.tensor.matmul(ps_c[:, 1536:2048], lhsT=lv1,
                                     rhs=et[:, 1536:2048],
                                     start=first, stop=last)

                # fast evict releases ctx psum banks; normalize from
                # SBUF in bf16 (2x DVE modes, half-size broadcast)
                cu = prr.tile([HD + 1, 2 * QB], BF, tag="cu", name=f"cu{it}")
                nc.vector.tensor_copy(cu[:], ps_c[:])
                rr = prr.tile([1, 2 * QB], BF, tag="rr", name=f"rr{it}")
                with nc.allow_low_precision(reason="softmax denominators "
                                            "tolerate bf16 reciprocal"):
                    nc.vector.reciprocal(rr[:], cu[HD:HD + 1, :])
                rrb = prr.tile([HD, 2 * QB], BF, tag="rrb", name=f"rrb{it}")
                nc.gpsimd.partition_broadcast(rrb[:], rr[:])
                nc.vector.tensor_mul(
                    ctxP[pair][0:HD, q0:q0 + QB],
                    cu[0:HD, 0:QB], rrb[0:HD, 0:QB])
                nc.vector.tensor_mul(
                    ctxP[pair][HD:P, q0:q0 + QB],
                    cu[0:HD, QB:2 * QB], rrb[0:HD, QB:2 * QB])

        lb.close()

        # ================= Phase C: output projection =================
        ppx = l0.enter_context(tc.tile_pool(name="psX", bufs=2, space="PSUM"))
        post = l0.enter_context(tc.tile_pool(name="post", bufs=2))

        for stg in range(4):
            ob = post.tile([P, 4 * D], BF, tag="ost", name=f"ob{stg}")
            for sti in range(4):
                st = stg * 4 + sti
                pso = ppx.tile([P, D], DT, tag="pse", name=f"pse{st}")
                for cc in range(4):
                    lhs = ctxP[cc][:, st * P:(st + 1) * P]
                    for h in range(2):
                        nc.tensor.matmul(pso[:, h * 512:(h + 1) * 512],
                                         lhsT=lhs,
                                         rhs=wo_t[cc][:, h * 512:(h + 1) * 512],
                                         start=(cc == 0), stop=(cc == 3))
                nc.scalar.copy(ob[:, sti * D:(sti + 1) * D], pso[:])
            nc.sync.dma_start(out_d[:, stg * 4 * D:(stg + 1) * 4 * D], ob[:])


_CACHED_NC = None


def _get_program():
    global _CACHED_NC
    if _CACHED_NC is None:
        nc = bacc.Bacc("TRN2", target_bir_lowering=False, debug=False,
                       num_devices=8)
        _emit_kernel(nc)
        nc.compile()
        _CACHED_NC = nc
    return _CACHED_NC


def _bf16(x):
    from ml_dtypes import bfloat16
    return np.ascontiguousarray(np.asarray(x, np.float32)).astype(bfloat16)


def _make_in_maps(input, wq, bq, wk, bk, wv, bv, wo, bo):
    input = np.asarray(input, np.float32)
    wqT_f = np.ascontiguousarray(np.asarray(wq, np.float32).T)
    wkT_f = np.ascontiguousarray(np.asarray(wk, np.float32).T)
    wvT_f = np.ascontiguousarray(np.asarray(wv, np.float32).T)
    woT_f = np.ascontiguousarray(np.asarray(wo, np.float32).T)
    bq = np.asarray(bq, np.float32)
    bk = np.asarray(bk, np.float32)
    bv = np.asarray(bv, np.float32)
    in_maps = []
    for core in range(8):
        b, g = core // 2, core % 2
        gsl = slice(g * GD, (g + 1) * GD)
        def _tiled(x):
            # [C*128, N] -> [128, C*N] with t[p, c*N+n] = x[c*128+p, n]
            c = x.shape[0] // P
            return x.reshape(c, P, -1).transpose(1, 0, 2).reshape(P, -1)
        in_maps.append({
            "inputT": _bf16(_tiled(input[b].T)),
            "wqT": _bf16(_tiled(wqT_f[:, gsl])),
            "wkT": _bf16(_tiled(wkT_f[:, gsl])),
            "wvT": _bf16(_tiled(wvT_f[:, gsl])),
            "woT": _bf16(_tiled(woT_f[gsl, :])),
            "bq": np.ascontiguousarray((bq[gsl] / 8.0).reshape(4, P).T),
            "bk": np.ascontiguousarray(bk[gsl].reshape(4, P).T),
            "bv": _bf16(bv[gsl].reshape(1, GD)),
        })
    return in_maps


def _combine(results, bo):
    bo = np.asarray(bo, np.float32)
    out = np.empty((BS, S, D), np.float32)
    for b in range(BS):
        def _untile(a):
            # [128, 16*D] -> [2048, D]: a[p, st*D+c] = out[st*128+p, c]
            return np.ascontiguousarray(
                a.reshape(P, 16, D).transpose(1, 0, 2).reshape(S, D))
        out[b] = (_untile(results[2 * b]["out"]).astype(np.float32)
                  + _untile(results[2 * b + 1]["out"]).astype(np.float32)
                  + bo)
    return out


def _numpy_fallback(input, mask, wq, bq, wk, bk, wv, bv, wo, bo):
    x = np.asarray(input, np.float32)
    bs, qlen, dim = x.shape
    def proj(w, b):
        y = x @ np.asarray(w, np.float32).T + np.asarray(b, np.float32)
        return y.reshape(bs, qlen, NH, HD).transpose(0, 2, 1, 3)
    q = proj(wq, bq) / np.sqrt(HD)
    k = proj(wk, bk)
    v = proj(wv, bv)
    scores = np.einsum("bhqd,bhkd->bhqk", q, k)
    pad = (np.asarray(mask) == 0)[:, None, None, :]
    scores = np.where(pad, -np.inf, scores)
    scores -= scores.max(axis=-1, keepdims=True)
    e = np.exp(scores)
    w8 = e / e.sum(axis=-1, keepdims=True)
    ctx = np.einsum("bhqk,bhkd->bhqd", w8, v)
    ctx = ctx.transpose(0, 2, 1, 3).reshape(bs, qlen, dim)
    return ctx @ np.asarray(wo, np.float32).T + np.asarray(bo, np.float32)


def run_on_device(inputs, trace=False, **trace_kwargs):
    """Returns (BassKernelResults, combined_output)."""
    nc = _get_program()
    in_maps = _make_in_maps(
        inputs["input"], inputs["wq"], inputs["bq"], inputs["wk"],
        inputs["bk"], inputs["wv"], inputs["bv"], inputs["wo"], inputs["bo"])
    res = bass_utils.run_bass_kernel_spmd(
        nc, in_maps, core_ids=list(range(8)), trace=trace, **trace_kwargs)
    out = _combine(res.results, inputs["bo"])
    return res, out


def kernel(**inputs) -> np.ndarray:
    mask = np.asarray(inputs["mask"])
    if not np.all(mask != 0):
        # fully general (masked) path; the shipped workload always has an
        # all-ones mask so this never triggers on-device sharding
        return _numpy_fallback(**inputs).astype(np.float32)
    _, out = run_on_device(inputs)
    return out


if __name__ == "__main__":
    rng = np.random.default_rng(0)
    ins = {
        "input": rng.normal(size=(BS, S, D)).astype(np.float32),
        "mask": np.ones((BS, S), np.int32),
        "wq": (rng.normal(size=(D, D)) * 0.02).astype(np.float32),
        "bq": (rng.normal(size=(D,)) * 0.02).astype(np.float32),
        "wk": (rng.normal(size=(D, D)) * 0.02).astype(np.float32),
        "bk": (rng.normal(size=(D,)) * 0.02).astype(np.float32),
        "wv": (rng.normal(size=(D, D)) * 0.02).astype(np.float32),
        "bv": (rng.normal(size=(D,)) * 0.02).astype(np.float32),
        "wo": (rng.normal(size=(D, D)) * 0.02).astype(np.float32),
        "bo": (rng.normal(size=(D,)) * 0.02).astype(np.float32),
    }
    out = kernel(**ins)
    exp = _numpy_fallback(**ins)
    err = np.abs(out - exp).max() / np.abs(exp).max()
    print("smoke rel err:", err)


# revision 13
# speedup vs baseline: 1.1612x; 1.1612x over previous
"""TRN2 Bass kernel for nn_MultiHeadAttention_82411832476301.

Full inputs in, full output out. Sharding: 8 cores = 4 batches x 2
head-groups (8 heads each). All matmul operands bf16 (PSUM accumulates
fp32). Design minimizes total engine work and DMA descriptor count
(measured: DMA is ~30-40ns/segment descriptor-bound; cross-engine
concurrency is weak, so evicts go to whichever engine idles in that
phase):

  - All DRAM tensors pre-tiled [128, C*N] so each DMA is one contiguous
    8-32KB descriptor per partition (13 DMAs total).
  - Phase A: Q/K projections as 32-MM psum chains into [128, 2048],
    ScalarE Identity-evict with fused bias (+1/8 scale for Q, bias
    pre-divided on host); V projection into vaug[st] [128, 520] with
    gpsimd-memset ones columns (65th col per head = softmax denominator
    trick); ScalarE strided copy-evict.
  - Phase B per (pair, q-block of 1024): per k-chunk, 4 score matmuls
    into one joint PSUM tile [128, 2048] (h0 cols 0-1023, h1 cols
    1024-2047), ONE exp activation N=2048 (best ScalarE overhead
    amortization), 4 context matmuls accumulating [65, 2048] joint ctx
    (row 64 = denominator). Tail: one fast DVE evict (frees psum),
    bf16 reciprocal + gpsimd partition_broadcast + 2x-mode DVE
    multiplies into ctxP.
  - Phase C: single output projection (4-pair accumulation), ScalarE
    evicts, 4 batched output DMAs, out bf16 in tiled layout.
Host combines: out[b] = untile(core(2b).out) + untile(core(2b+1).out)
+ bo (fp32).
"""

import sys

if "/opt/trn_rl_repo" not in sys.path:
    sys.path.insert(0, "/opt/trn_rl_repo")

import numpy as np
from contextlib import ExitStack

import concourse.bass as bass
import concourse.mybir as mybir
import concourse.tile as tile
from concourse import bacc
from concourse import bass_utils

P = 128
BS = 4
S = 2048          # sequence length
D = 1024          # model dim
NH = 16           # total heads
HD = 64           # head dim
G = 8             # heads per group (per core)
GD = G * HD       # 512 dims per group
GDA = G * (HD + 1)  # 520: v-dims augmented with ones columns
QB = 1024         # q block size
NQB = S // QB     # 2
KT = S // P       # 16 k-chunks of 128
DT = mybir.dt.float32
BF = mybir.dt.bfloat16
FP = mybir.ActivationFunctionType
ALU = mybir.AluOpType


def _emit_kernel(nc):
    # tiled layouts: x_tiled[p, c*N + n] = x[c*128 + p, n] -> per-partition
    # rows are one contiguous DMA descriptor each
    inputT = nc.dram_tensor("inputT", (P, 8 * S), BF, kind="ExternalInput").ap()
    wqT = nc.dram_tensor("wqT", (P, 8 * GD), BF, kind="ExternalInput").ap()
    wkT = nc.dram_tensor("wkT", (P, 8 * GD), BF, kind="ExternalInput").ap()
    wvT = nc.dram_tensor("wvT", (P, 8 * GD), BF, kind="ExternalInput").ap()
    woT = nc.dram_tensor("woT", (P, 4 * D), BF, kind="ExternalInput").ap()
    bq_d = nc.dram_tensor("bq", (P, 4), DT, kind="ExternalInput").ap()
    bk_d = nc.dram_tensor("bk", (P, 4), DT, kind="ExternalInput").ap()
    bv_d = nc.dram_tensor("bv", (1, GD), BF, kind="ExternalInput").ap()
    out_d = nc.dram_tensor("out", (P, 16 * D), BF, kind="ExternalOutput").ap()

    with tile.TileContext(nc) as tc:
        _body(nc, tc, inputT, wqT, wkT, wvT, woT, bq_d, bk_d, bv_d,
              out_d)
    return nc


def _body(nc, tc, inputT, wqT, wkT, wvT, woT, bq_d, bk_d, bv_d,
          out_d):
    with ExitStack() as l0:
        pconst = l0.enter_context(tc.tile_pool(name="const", bufs=1))
        pqkv = l0.enter_context(tc.tile_pool(name="qkv", bufs=1))

        ones_t = pconst.tile([1, P], BF, tag="ones", name="ones_t")
        nc.gpsimd.memset(ones_t[:], 1.0)
        bq_sb = pconst.tile([P, 4], DT, tag="bq", name="bq_sb")
        nc.sync.dma_start(bq_sb[:], bq_d[:])
        bk_sb = pconst.tile([P, 4], DT, tag="bk", name="bk_sb")
        nc.sync.dma_start(bk_sb[:], bk_d[:])
        bv_sb = pconst.tile([1, GD], BF, tag="bv", name="bv_sb")
        nc.sync.dma_start(bv_sb[:], bv_d[:])

        qT = [pqkv.tile([P, S], BF, tag=f"q{ec}", name=f"qT{ec}")
              for ec in range(4)]
        kT = [pqkv.tile([P, S], BF, tag=f"k{ec}", name=f"kT{ec}")
              for ec in range(4)]
        vaug = [pqkv.tile([P, GDA], BF, tag=f"v{st}", name=f"vaug{st}")
                for st in range(KT)]

        # ================= Phase A: projections =================
        with ExitStack() as la:
            pin = la.enter_context(tc.tile_pool(name="pin", bufs=1))
            pw = la.enter_context(tc.tile_pool(name="pw", bufs=1))
            ppv = la.enter_context(
                tc.tile_pool(name="psAV", bufs=2, space="PSUM"))
            ppa = la.enter_context(
                tc.tile_pool(name="psA", bufs=1, space="PSUM"))

            # s-half-major input layout: in_big[p, h*8192 + dc*1024 + s]
            # = input.T[dc*128+p, h*1024+s]; two half-DMAs so the first
            # projections unblock after 2MB instead of 4MB
            in_big = pin.tile([P, 8 * S], BF, tag="inb", name="in_big")
            nc.sync.dma_start(in_big[:, 0:8 * QB], inputT[:, 0:8 * QB])
            nc.sync.dma_start(in_big[:, 8 * QB:16 * QB],
                              inputT[:, 8 * QB:16 * QB])

            def int_sl(dc, s0, width):
                h, sh = s0 // QB, s0 % QB
                base = h * 8 * QB + dc * QB + sh
                return in_big[:, base:base + width]
            wv_big = pw.tile([P, 8 * GD], BF, tag="wvb", name="wv_big")
            nc.sync.dma_start(wv_big[:], wvT[:])
            wv_t = [wv_big[:, dc * GD:(dc + 1) * GD] for dc in range(8)]
            wst = {}
            for p, wdram in enumerate((wqT, wkT)):
                big = pw.tile([P, 8 * GD], BF, tag=f"wb{p}", name=f"w_big{p}")
                nc.sync.dma_start(big[:], wdram[:])
                for dc in range(8):
                    wst[p, dc] = big[:, dc * GD:(dc + 1) * GD]

            # V projection (512-wide); ones columns set by gpsimd memset
            for st in range(KT):
                ones_cols = vaug[st][:].rearrange(
                    "p (h c) -> p h c", c=HD + 1)[:, :, HD:HD + 1]
                nc.gpsimd.memset(ones_cols, 1.0)
                ps = ppv.tile([P, GD], DT, tag="psv", name=f"psV{st}")
                for dc in range(8):
                    lhs = int_sl(dc, st * P, P)
                    nc.tensor.matmul(ps[:], lhsT=lhs, rhs=wv_t[dc],
                                     start=(dc == 0), stop=False)
                nc.tensor.matmul(ps[:], lhsT=ones_t[0:1, 0:P],
                                 rhs=bv_sb[0:1, :],
                                 start=False, stop=True)
                dst3 = vaug[st][:].rearrange(
                    "p (h c) -> p h c", c=HD + 1)[:, :, 0:HD]
                nc.scalar.copy(dst3, ps[:].rearrange(
                    "p (h c) -> p h c", c=HD))

            # Q/K projections: full-row psum chains, one evict per pair
            for p in range(2):
                for ec in range(4):
                    ps = ppa.tile([P, S], DT, tag="psqk",
                                  name=f"psA{p}_{ec}")
                    for sb in range(4):
                        sl = slice(sb * 512, sb * 512 + 512)
                        for dc in range(8):
                            nc.tensor.matmul(
                                ps[:, sl],
                                lhsT=wst[p, dc][:, ec * P:(ec + 1) * P],
                                rhs=int_sl(dc, sb * 512, 512),
                                start=(dc == 0), stop=(dc == 7))
                    dest = (qT if p == 0 else kT)[ec][:]
                    bias = (bq_sb if p == 0 else bk_sb)[:, ec:ec + 1]
                    nc.scalar.activation(
                        dest, ps[:], FP.Identity, bias=bias,
                        scale=(0.125 if p == 0 else 1.0))

        # prefetch output-projection weights (DMA idle during attention)
        pwo = l0.enter_context(tc.tile_pool(name="pwo", bufs=1))
        wo_big = pwo.tile([P, 4 * D], BF, tag="wob", name="wo_big")
        nc.sync.dma_start(wo_big[:], woT[:])
        wo_t = [wo_big[:, cc * D:(cc + 1) * D] for cc in range(4)]

        # ================= Phase B: attention =================
        pctx = l0.enter_context(tc.tile_pool(name="ctxp", bufs=1))
        ctxP = [pctx.tile([P, S], BF, tag=f"ctx{cc}", name=f"ctxP{cc}")
                for cc in range(4)]

        lb = ExitStack()
        pet = lb.enter_context(tc.tile_pool(name="et", bufs=2))
        prr = lb.enter_context(tc.tile_pool(name="rr", bufs=2))
        pps = lb.enter_context(tc.tile_pool(name="psS", bufs=1, space="PSUM"))
        ppc = lb.enter_context(tc.tile_pool(name="psC", bufs=1, space="PSUM"))

        for pair in range(4):
            for qb in range(NQB):
                it = pair * NQB + qb
                q0 = qb * QB
                ps_s = pps.tile([P, 2 * QB], DT, tag="pss", name=f"pss{it}")
                ps_c = ppc.tile([HD + 1, 2 * QB], DT, tag="psc",
                                name=f"psc{it}")
                for kt in range(KT):
                    lk0 = kT[pair][0:HD, kt * P:(kt + 1) * P]
                    lk1 = kT[pair][HD:P, kt * P:(kt + 1) * P]
                    first, last = kt == 0, kt == KT - 1
                    # scores: h0 -> cols 0:1024, h1 -> cols 1024:2048
                    nc.tensor.matmul(ps_s[:, 0:512], lhsT=lk0,
                                     rhs=qT[pair][0:HD, q0:q0 + 512],
                                     start=True, stop=True)
                    nc.tensor.matmul(ps_s[:, 512:1024], lhsT=lk0,
                                     rhs=qT[pair][0:HD, q0 + 512:q0 + QB],
                                     start=True, stop=True)
                    nc.tensor.matmul(ps_s[:, 1024:1536], lhsT=lk1,
                                     rhs=qT[pair][HD:P, q0:q0 + 512],
                                     start=True, stop=True)
                    nc.tensor.matmul(ps_s[:, 1536:2048], lhsT=lk1,
                                     rhs=qT[pair][HD:P, q0 + 512:q0 + QB],
                                     start=True, stop=True)
                    et = pet.tile([P, 2 * QB], BF, tag="et",
                                  name=f"et{it}_{kt}")
                    nc.scalar.activation(et[:], ps_s[:], FP.Exp)
                    # context accumulation (65-row: row 64 = denominator)
                    lv0 = vaug[kt][:, (2 * pair) * 65:(2 * pair) * 65 + 65]
                    lv1 = vaug[kt][:, (2 * pair + 1) * 65:
                                   (2 * pair + 1) * 65 + 65]
                    nc.tensor.matmul(ps_c[:, 0:512], lhsT=lv0,
                                     rhs=et[:, 0:512],
                                     start=first, stop=last)
                    nc.tensor.matmul(ps_c[:, 512:1024], lhsT=lv0,
                                     rhs=et[:, 512:1024],
                                     start=first, stop=last)
                    nc.tensor.matmul(ps_c[:, 1024:1536], lhsT=lv1,
                                     rhs=et[:, 1024:1536],
                                     start=first, stop=last)
                    nc.tensor.matmul(ps_c[:, 1536:2048], lhsT=lv1,
                                     rhs=et[:, 1536:2048],
                                     start=first, stop=last)

                # fast evict releases ctx psum banks; normalize from
                # SBUF in bf16 (2x DVE modes, half-size broadcast)
                cu = prr.tile([HD + 1, 2 * QB], BF, tag="cu", name=f"cu{it}")
                nc.vector.tensor_copy(cu[:], ps_c[:])
                rr = prr.tile([1, 2 * QB], BF, tag="rr", name=f"rr{it}")
                with nc.allow_low_precision(reason="softmax denominators "
                                            "tolerate bf16 reciprocal"):
                    nc.vector.reciprocal(rr[:], cu[HD:HD + 1, :])
                rrb = prr.tile([HD, 2 * QB], BF, tag="rrb", name=f"rrb{it}")
                nc.gpsimd.partition_broadcast(rrb[:], rr[:])
                nc.vector.tensor_mul(
                    ctxP[pair][0:HD, q0:q0 + QB],
                    cu[0:HD, 0:QB], rrb[0:HD, 0:QB])
                nc.vector.tensor_mul(
                    ctxP[pair][HD:P, q0:q0 + QB],
                    cu[0:HD, QB:2 * QB], rrb[0:HD, QB:2 * QB])

        lb.close()

        # ================= Phase C: output projection =================
        ppx = l0.enter_context(tc.tile_pool(name="psX", bufs=2, space="PSUM"))
        post = l0.enter_context(tc.tile_pool(name="post", bufs=2))

        for stg in range(4):
            ob = post.tile([P, 4 * D], BF, tag="ost", name=f"ob{stg}")
            for sti in range(4):
                st = stg * 4 + sti
                pso = ppx.tile([P, D], DT, tag="pse", name=f"pse{st}")
                for cc in range(4):
                    lhs = ctxP[cc][:, st * P:(st + 1) * P]
                    for h in range(2):
                        nc.tensor.matmul(pso[:, h * 512:(h + 1) * 512],
                                         lhsT=lhs,
                                         rhs=wo_t[cc][:, h * 512:(h + 1) * 512],
                                         start=(cc == 0), stop=(cc == 3))
                nc.scalar.copy(ob[:, sti * D:(sti + 1) * D], pso[:])
            nc.sync.dma_start(out_d[:, stg * 4 * D:(stg + 1) * 4 * D], ob[:])


_CACHED_NC = None


def _get_program():
    global _CACHED_NC
    if _CACHED_NC is None:
        nc = bacc.Bacc("TRN2", target_bir_lowering=False, debug=False,
                       num_devices=8)
        _emit_kernel(nc)
        nc.compile()
        _CACHED_NC = nc
    return _CACHED_NC


def _bf16(x):
    from ml_dtypes import bfloat16
    return np.ascontiguousarray(np.asarray(x, np.float32)).astype(bfloat16)


def _make_in_maps(input, wq, bq, wk, bk, wv, bv, wo, bo):
    input = np.asarray(input, np.float32)
    wqT_f = np.ascontiguousarray(np.asarray(wq, np.float32).T)
    wkT_f = np.ascontiguousarray(np.asarray(wk, np.float32).T)
    wvT_f = np.ascontiguousarray(np.asarray(wv, np.float32).T)
    woT_f = np.ascontiguousarray(np.asarray(wo, np.float32).T)
    bq = np.asarray(bq, np.float32)
    bk = np.asarray(bk, np.float32)
    bv = np.asarray(bv, np.float32)
    in_maps = []
    for core in range(8):
        b, g = core // 2, core % 2
        gsl = slice(g * GD, (g + 1) * GD)
        def _tiled(x):
            # [C*128, N] -> [128, C*N] with t[p, c*N+n] = x[c*128+p, n]
            c = x.shape[0] // P
            return x.reshape(c, P, -1).transpose(1, 0, 2).reshape(P, -1)
        xt = np.ascontiguousarray(input[b].T)  # [1024, 2048]
        in2 = (xt.reshape(8, P, 2, QB).transpose(2, 1, 0, 3)
               .reshape(2, P, 8 * QB).transpose(1, 0, 2).reshape(P, 16 * QB))
        in_maps.append({
            "inputT": _bf16(in2),
            "wqT": _bf16(_tiled(wqT_f[:, gsl])),
            "wkT": _bf16(_tiled(wkT_f[:, gsl])),
            "wvT": _bf16(_tiled(wvT_f[:, gsl])),
            "woT": _bf16(_tiled(woT_f[gsl, :])),
            "bq": np.ascontiguousarray((bq[gsl] / 8.0).reshape(4, P).T),
            "bk": np.ascontiguousarray(bk[gsl].reshape(4, P).T),
            "bv": _bf16(bv[gsl].reshape(1, GD)),
        })
    return in_maps


def _combine(results, bo):
    bo = np.asarray(bo, np.float32)
    out = np.empty((BS, S, D), np.float32)
    for b in range(BS):
        def _untile(a):
            # [128, 16*D] -> [2048, D]: a[p, st*D+c] = out[st*128+p, c]
            return np.ascontiguousarray(
                a.reshape(P, 16, D).transpose(1, 0, 2).reshape(S, D))
        out[b] = (_untile(results[2 * b]["out"]).astype(np.float32)
                  + _untile(results[2 * b + 1]["out"]).astype(np.float32)
                  + bo)
    return out


def _numpy_fallback(input, mask, wq, bq, wk, bk, wv, bv, wo, bo):
    x = np.asarray(input, np.float32)
    bs, qlen, dim = x.shape
    def proj(w, b):
        y = x @ np.asarray(w, np.float32).T + np.asarray(b, np.float32)
        return y.reshape(bs, qlen, NH, HD).transpose(0, 2, 1, 3)
    q = proj(wq, bq) / np.sqrt(HD)
    k = proj(wk, bk)
    v = proj(wv, bv)
    scores = np.einsum("bhqd,bhkd->bhqk", q, k)
    pad = (np.asarray(mask) == 0)[:, None, None, :]
    scores = np.where(pad, -np.inf, scores)
    scores -= scores.max(axis=-1, keepdims=True)
    e = np.exp(scores)
    w8 = e / e.sum(axis=-1, keepdims=True)
    ctx = np.einsum("bhqk,bhkd->bhqd", w8, v)
    ctx = ctx.transpose(0, 2, 1, 3).reshape(bs, qlen, dim)
    return ctx @ np.asarray(wo, np.float32).T + np.asarray(bo, np.float32)


def run_on_device(inputs, trace=False, **trace_kwargs):
    """Returns (BassKernelResults, combined_output)."""
    nc = _get_program()
    in_maps = _make_in_maps(
        inputs["input"], inputs["wq"], inputs["bq"], inputs["wk"],
        inputs["bk"], inputs["wv"], inputs["bv"], inputs["wo"], inputs["bo"])
    res = bass_utils.run_bass_kernel_spmd(
        nc, in_maps, core_ids=list(range(8)), trace=trace, **trace_kwargs)
    out = _combine(res.results, inputs["bo"])
    return res, out


def kernel(**inputs) -> np.ndarray:
    mask = np.asarray(inputs["mask"])
    if not np.all(mask != 0):
        # fully general (masked) path; the shipped workload always has an
        # all-ones mask so this never triggers on-device sharding
        return _numpy_fallback(**inputs).astype(np.float32)
    _, out = run_on_device(inputs)
    return out


if __name__ == "__main__":
    rng = np.random.default_rng(0)
    ins = {
        "input": rng.normal(size=(BS, S, D)).astype(np.float32),
        "mask": np.ones((BS, S), np.int32),
        "wq": (rng.normal(size=(D, D)) * 0.02).astype(np.float32),
        "bq": (rng.normal(size=(D,)) * 0.02).astype(np.float32),
        "wk": (rng.normal(size=(D, D)) * 0.02).astype(np.float32),
        "bk": (rng.normal(size=(D,)) * 0.02).astype(np.float32),
        "wv": (rng.normal(size=(D, D)) * 0.02).astype(np.float32),
        "bv": (rng.normal(size=(D,)) * 0.02).astype(np.float32),
        "wo": (rng.normal(size=(D, D)) * 0.02).astype(np.float32),
        "bo": (rng.normal(size=(D,)) * 0.02).astype(np.float32),
    }
    out = kernel(**ins)
    exp = _numpy_fallback(**ins)
    err = np.abs(out - exp).max() / np.abs(exp).max()
    print("smoke rel err:", err)


# revision 14
# speedup vs baseline: 1.2859x; 1.1074x over previous
"""TRN2 Bass kernel for nn_MultiHeadAttention_82411832476301.

Full inputs in, full output out. Sharding: 8 cores = 4 batches x 2
head-groups (8 heads each). All matmul operands bf16 (PSUM accumulates
fp32). Design minimizes total engine work and DMA descriptor count
(measured: DMA is ~30-40ns/segment descriptor-bound; cross-engine
concurrency is weak, so evicts go to whichever engine idles in that
phase):

  - All DRAM tensors pre-tiled [128, C*N] so each DMA is one contiguous
    8-32KB descriptor per partition (13 DMAs total).
  - Phase A: Q/K projections as 32-MM psum chains into [128, 2048],
    ScalarE Identity-evict with fused bias (+1/8 scale for Q, bias
    pre-divided on host); V projection into vaug[st] [128, 520] with
    gpsimd-memset ones columns (65th col per head = softmax denominator
    trick); ScalarE strided copy-evict.
  - Phase B per (pair, q-block of 1024): per k-chunk, 4 score matmuls
    into one joint PSUM tile [128, 2048] (h0 cols 0-1023, h1 cols
    1024-2047), ONE exp activation N=2048 (best ScalarE overhead
    amortization), 4 context matmuls accumulating [65, 2048] joint ctx
    (row 64 = denominator). Tail: one fast DVE evict (frees psum),
    bf16 reciprocal + gpsimd partition_broadcast + 2x-mode DVE
    multiplies into ctxP.
  - Phase C: single output projection (4-pair accumulation), ScalarE
    evicts, 4 batched output DMAs, out bf16 in tiled layout.
Host combines: out[b] = untile(core(2b).out) + untile(core(2b+1).out)
+ bo (fp32).
"""

import sys

if "/opt/trn_rl_repo" not in sys.path:
    sys.path.insert(0, "/opt/trn_rl_repo")

import numpy as np
from contextlib import ExitStack

import concourse.bass as bass
import concourse.mybir as mybir
import concourse.tile as tile
from concourse import bacc
from concourse import bass_utils

P = 128
BS = 4
S = 2048          # sequence length
D = 1024          # model dim
NH = 16           # total heads
HD = 64           # head dim
G = 8             # heads per group (per core)
GD = G * HD       # 512 dims per group
GDA = G * (HD + 1)  # 520: v-dims augmented with ones columns
QB = 1024         # q block size
NQB = S // QB     # 2
KT = S // P       # 16 k-chunks of 128
DT = mybir.dt.float32
BF = mybir.dt.bfloat16
FP = mybir.ActivationFunctionType
ALU = mybir.AluOpType


def _emit_kernel(nc):
    # tiled layouts: x_tiled[p, c*N + n] = x[c*128 + p, n] -> per-partition
    # rows are one contiguous DMA descriptor each
    inputT = nc.dram_tensor("inputT", (P, 8 * S), BF, kind="ExternalInput").ap()
    wqT = nc.dram_tensor("wqT", (P, 8 * GD), BF, kind="ExternalInput").ap()
    wkT = nc.dram_tensor("wkT", (P, 8 * GD), BF, kind="ExternalInput").ap()
    wvT = nc.dram_tensor("wvT", (P, 8 * GD), BF, kind="ExternalInput").ap()
    woT = nc.dram_tensor("woT", (P, 4 * D), BF, kind="ExternalInput").ap()
    bq_d = nc.dram_tensor("bq", (P, 4), DT, kind="ExternalInput").ap()
    bk_d = nc.dram_tensor("bk", (P, 4), DT, kind="ExternalInput").ap()
    bv_d = nc.dram_tensor("bv", (1, GD), BF, kind="ExternalInput").ap()
    out_d = nc.dram_tensor("out", (P, 16 * D), BF, kind="ExternalOutput").ap()

    with tile.TileContext(nc) as tc:
        _body(nc, tc, inputT, wqT, wkT, wvT, woT, bq_d, bk_d, bv_d,
              out_d)
    return nc


def _body(nc, tc, inputT, wqT, wkT, wvT, woT, bq_d, bk_d, bv_d,
          out_d):
    with ExitStack() as l0:
        pconst = l0.enter_context(tc.tile_pool(name="const", bufs=1))
        pqkv = l0.enter_context(tc.tile_pool(name="qkv", bufs=1))

        ones_t = pconst.tile([1, P], BF, tag="ones", name="ones_t")
        nc.gpsimd.memset(ones_t[:], 1.0)
        bq_sb = pconst.tile([P, 4], DT, tag="bq", name="bq_sb")
        nc.sync.dma_start(bq_sb[:], bq_d[:])
        bk_sb = pconst.tile([P, 4], DT, tag="bk", name="bk_sb")
        nc.sync.dma_start(bk_sb[:], bk_d[:])
        bv_sb = pconst.tile([1, GD], BF, tag="bv", name="bv_sb")
        nc.sync.dma_start(bv_sb[:], bv_d[:])

        qT = [pqkv.tile([P, S], BF, tag=f"q{ec}", name=f"qT{ec}")
              for ec in range(4)]
        kT = [pqkv.tile([P, S], BF, tag=f"k{ec}", name=f"kT{ec}")
              for ec in range(4)]
        vaug = [pqkv.tile([P, GDA], BF, tag=f"v{st}", name=f"vaug{st}")
                for st in range(KT)]

        # ================= Phase A: projections =================
        with ExitStack() as la:
            pin = la.enter_context(tc.tile_pool(name="pin", bufs=1))
            pw = la.enter_context(tc.tile_pool(name="pw", bufs=1))

            # s-half-major input layout: in_big[p, h*8192 + dc*1024 + s]
            # = input.T[dc*128+p, h*1024+s]; two half-DMAs so the first
            # projections unblock after 2MB instead of 4MB
            in_big = pin.tile([P, 8 * S], BF, tag="inb", name="in_big")
            nc.sync.dma_start(in_big[:, 0:8 * QB], inputT[:, 0:8 * QB])
            nc.sync.dma_start(in_big[:, 8 * QB:16 * QB],
                              inputT[:, 8 * QB:16 * QB])

            def int_sl(dc, s0, width):
                h, sh = s0 // QB, s0 % QB
                base = h * 8 * QB + dc * QB + sh
                return in_big[:, base:base + width]
            wv_big = pw.tile([P, 8 * GD], BF, tag="wvb", name="wv_big")
            nc.sync.dma_start(wv_big[:], wvT[:])
            wv_t = [wv_big[:, dc * GD:(dc + 1) * GD] for dc in range(8)]
            wst = {}
            for p, wdram in enumerate((wqT, wkT)):
                big = pw.tile([P, 8 * GD], BF, tag=f"wb{p}", name=f"w_big{p}")
                nc.sync.dma_start(big[:], wdram[:])
                for dc in range(8):
                    wst[p, dc] = big[:, dc * GD:(dc + 1) * GD]

            # V projection (512-wide); ones columns set by gpsimd memset
            lv_ = ExitStack()
            ppv = lv_.enter_context(
                tc.tile_pool(name="psAV", bufs=2, space="PSUM"))
            for st in range(KT):
                ones_cols = vaug[st][:].rearrange(
                    "p (h c) -> p h c", c=HD + 1)[:, :, HD:HD + 1]
                nc.gpsimd.memset(ones_cols, 1.0)
                ps = ppv.tile([P, GD], DT, tag="psv", name=f"psV{st}")
                for dc in range(8):
                    lhs = int_sl(dc, st * P, P)
                    nc.tensor.matmul(ps[:], lhsT=lhs, rhs=wv_t[dc],
                                     start=(dc == 0), stop=False)
                nc.tensor.matmul(ps[:], lhsT=ones_t[0:1, 0:P],
                                 rhs=bv_sb[0:1, :],
                                 start=False, stop=True)
                dst3 = vaug[st][:].rearrange(
                    "p (h c) -> p h c", c=HD + 1)[:, :, 0:HD]
                nc.scalar.copy(dst3, ps[:].rearrange(
                    "p (h c) -> p h c", c=HD))

            lv_.close()

            # Q/K projections: double-buffered full-row psum chains
            ppa = la.enter_context(
                tc.tile_pool(name="psA", bufs=2, space="PSUM"))
            for p in range(2):
                for ec in range(4):
                    ps = ppa.tile([P, S], DT, tag="psqk",
                                  name=f"psA{p}_{ec}")
                    for sb in range(4):
                        sl = slice(sb * 512, sb * 512 + 512)
                        for dc in range(8):
                            nc.tensor.matmul(
                                ps[:, sl],
                                lhsT=wst[p, dc][:, ec * P:(ec + 1) * P],
                                rhs=int_sl(dc, sb * 512, 512),
                                start=(dc == 0), stop=(dc == 7))
                    dest = (qT if p == 0 else kT)[ec][:]
                    bias = (bq_sb if p == 0 else bk_sb)[:, ec:ec + 1]
                    nc.scalar.activation(
                        dest, ps[:], FP.Identity, bias=bias,
                        scale=(0.125 if p == 0 else 1.0))

        # prefetch output-projection weights (DMA idle during attention)
        pwo = l0.enter_context(tc.tile_pool(name="pwo", bufs=1))
        wo_big = pwo.tile([P, 4 * D], BF, tag="wob", name="wo_big")
        nc.sync.dma_start(wo_big[:], woT[:])
        wo_t = [wo_big[:, cc * D:(cc + 1) * D] for cc in range(4)]

        # ================= Phase B: attention =================
        pctx = l0.enter_context(tc.tile_pool(name="ctxp", bufs=1))
        ctxP = [pctx.tile([P, S], BF, tag=f"ctx{cc}", name=f"ctxP{cc}")
                for cc in range(4)]

        lb = ExitStack()
        pet = lb.enter_context(tc.tile_pool(name="et", bufs=2))
        prr = lb.enter_context(tc.tile_pool(name="rr", bufs=2))
        pps = lb.enter_context(tc.tile_pool(name="psS", bufs=1, space="PSUM"))
        ppc = lb.enter_context(tc.tile_pool(name="psC", bufs=1, space="PSUM"))

        for pair in range(4):
            for qb in range(NQB):
                it = pair * NQB + qb
                q0 = qb * QB
                ps_s = pps.tile([P, 2 * QB], DT, tag="pss", name=f"pss{it}")
                ps_c = ppc.tile([HD + 1, 2 * QB], DT, tag="psc",
                                name=f"psc{it}")
                for kt in range(KT):
                    lk0 = kT[pair][0:HD, kt * P:(kt + 1) * P]
                    lk1 = kT[pair][HD:P, kt * P:(kt + 1) * P]
                    first, last = kt == 0, kt == KT - 1
                    # scores: h0 -> cols 0:1024, h1 -> cols 1024:2048
                    nc.tensor.matmul(ps_s[:, 0:512], lhsT=lk0,
                                     rhs=qT[pair][0:HD, q0:q0 + 512],
                                     start=True, stop=True)
                    nc.tensor.matmul(ps_s[:, 512:1024], lhsT=lk0,
                                     rhs=qT[pair][0:HD, q0 + 512:q0 + QB],
                                     start=True, stop=True)
                    nc.tensor.matmul(ps_s[:, 1024:1536], lhsT=lk1,
                                     rhs=qT[pair][HD:P, q0:q0 + 512],
                                     start=True, stop=True)
                    nc.tensor.matmul(ps_s[:, 1536:2048], lhsT=lk1,
                                     rhs=qT[pair][HD:P, q0 + 512:q0 + QB],
                                     start=True, stop=True)
                    et = pet.tile([P, 2 * QB], BF, tag="et",
                                  name=f"et{it}_{kt}")
                    nc.scalar.activation(et[:], ps_s[:], FP.Exp)
                    # context accumulation (65-row: row 64 = denominator)
                    lv0 = vaug[kt][:, (2 * pair) * 65:(2 * pair) * 65 + 65]
                    lv1 = vaug[kt][:, (2 * pair + 1) * 65:
                                   (2 * pair + 1) * 65 + 65]
                    nc.tensor.matmul(ps_c[:, 0:512], lhsT=lv0,
                                     rhs=et[:, 0:512],
                                     start=first, stop=last)
                    nc.tensor.matmul(ps_c[:, 512:1024], lhsT=lv0,
                                     rhs=et[:, 512:1024],
                                     start=first, stop=last)
                    nc.tensor.matmul(ps_c[:, 1024:1536], lhsT=lv1,
                                     rhs=et[:, 1024:1536],
                                     start=first, stop=last)
                    nc.tensor.matmul(ps_c[:, 1536:2048], lhsT=lv1,
                                     rhs=et[:, 1536:2048],
                                     start=first, stop=last)

                # fast evict releases ctx psum banks; normalize from
                # SBUF in bf16 (2x DVE modes, half-size broadcast)
                cu = prr.tile([HD + 1, 2 * QB], BF, tag="cu", name=f"cu{it}")
                nc.vector.tensor_copy(cu[:], ps_c[:])
                rr = prr.tile([1, 2 * QB], BF, tag="rr", name=f"rr{it}")
                with nc.allow_low_precision(reason="softmax denominators "
                                            "tolerate bf16 reciprocal"):
                    nc.vector.reciprocal(rr[:], cu[HD:HD + 1, :])
                rrb = prr.tile([HD, 2 * QB], BF, tag="rrb", name=f"rrb{it}")
                nc.gpsimd.partition_broadcast(rrb[:], rr[:])
                nc.vector.tensor_mul(
                    ctxP[pair][0:HD, q0:q0 + QB],
                    cu[0:HD, 0:QB], rrb[0:HD, 0:QB])
                nc.vector.tensor_mul(
                    ctxP[pair][HD:P, q0:q0 + QB],
                    cu[0:HD, QB:2 * QB], rrb[0:HD, QB:2 * QB])

        lb.close()

        # ================= Phase C: output projection =================
        ppx = l0.enter_context(tc.tile_pool(name="psX", bufs=2, space="PSUM"))
        post = l0.enter_context(tc.tile_pool(name="post", bufs=2))

        for stg in range(4):
            ob = post.tile([P, 4 * D], BF, tag="ost", name=f"ob{stg}")
            for sti in range(4):
                st = stg * 4 + sti
                pso = ppx.tile([P, D], DT, tag="pse", name=f"pse{st}")
                for cc in range(4):
                    lhs = ctxP[cc][:, st * P:(st + 1) * P]
                    for h in range(2):
                        nc.tensor.matmul(pso[:, h * 512:(h + 1) * 512],
                                         lhsT=lhs,
                                         rhs=wo_t[cc][:, h * 512:(h + 1) * 512],
                                         start=(cc == 0), stop=(cc == 3))
                nc.scalar.copy(ob[:, sti * D:(sti + 1) * D], pso[:])
            nc.sync.dma_start(out_d[:, stg * 4 * D:(stg + 1) * 4 * D], ob[:])


_CACHED_NC = None


def _get_program():
    global _CACHED_NC
    if _CACHED_NC is None:
        nc = bacc.Bacc("TRN2", target_bir_lowering=False, debug=False,
                       num_devices=8)
        _emit_kernel(nc)
        nc.compile()
        _CACHED_NC = nc
    return _CACHED_NC


def _bf16(x):
    from ml_dtypes import bfloat16
    return np.ascontiguousarray(np.asarray(x, np.float32)).astype(bfloat16)


def _make_in_maps(input, wq, bq, wk, bk, wv, bv, wo, bo):
    input = np.asarray(input, np.float32)
    wqT_f = np.ascontiguousarray(np.asarray(wq, np.float32).T)
    wkT_f = np.ascontiguousarray(np.asarray(wk, np.float32).T)
    wvT_f = np.ascontiguousarray(np.asarray(wv, np.float32).T)
    woT_f = np.ascontiguousarray(np.asarray(wo, np.float32).T)
    bq = np.asarray(bq, np.float32)
    bk = np.asarray(bk, np.float32)
    bv = np.asarray(bv, np.float32)
    in_maps = []
    for core in range(8):
        b, g = core // 2, core % 2
        gsl = slice(g * GD, (g + 1) * GD)
        def _tiled(x):
            # [C*128, N] -> [128, C*N] with t[p, c*N+n] = x[c*128+p, n]
            c = x.shape[0] // P
            return x.reshape(c, P, -1).transpose(1, 0, 2).reshape(P, -1)
        xt = np.ascontiguousarray(input[b].T)  # [1024, 2048]
        in2 = (xt.reshape(8, P, 2, QB).transpose(2, 1, 0, 3)
               .reshape(2, P, 8 * QB).transpose(1, 0, 2).reshape(P, 16 * QB))
        in_maps.append({
            "inputT": _bf16(in2),
            "wqT": _bf16(_tiled(wqT_f[:, gsl])),
            "wkT": _bf16(_tiled(wkT_f[:, gsl])),
            "wvT": _bf16(_tiled(wvT_f[:, gsl])),
            "woT": _bf16(_tiled(woT_f[gsl, :])),
            "bq": np.ascontiguousarray((bq[gsl] / 8.0).reshape(4, P).T),
            "bk": np.ascontiguousarray(bk[gsl].reshape(4, P).T),
            "bv": _bf16(bv[gsl].reshape(1, GD)),
        })
    return in_maps


def _combine(results, bo):
    bo = np.asarray(bo, np.float32)
    out = np.empty((BS, S, D), np.float32)
    for b in range(BS):
        def _untile(a):
            # [128, 16*D] -> [2048, D]: a[p, st*D+c] = out[st*128+p, c]
            return np.ascontiguousarray(
                a.reshape(P, 16, D).transpose(1, 0, 2).reshape(S, D))
        out[b] = (_untile(results[2 * b]["out"]).astype(np.float32)
                  + _untile(results[2 * b + 1]["out"]).astype(np.float32)
                  + bo)
    return out


def _numpy_fallback(input, mask, wq, bq, wk, bk, wv, bv, wo, bo):
    x = np.asarray(input, np.float32)
    bs, qlen, dim = x.shape
    def proj(w, b):
        y = x @ np.asarray(w, np.float32).T + np.asarray(b, np.float32)
        return y.reshape(bs, qlen, NH, HD).transpose(0, 2, 1, 3)
    q = proj(wq, bq) / np.sqrt(HD)
    k = proj(wk, bk)
    v = proj(wv, bv)
    scores = np.einsum("bhqd,bhkd->bhqk", q, k)
    pad = (np.asarray(mask) == 0)[:, None, None, :]
    scores = np.where(pad, -np.inf, scores)
    scores -= scores.max(axis=-1, keepdims=True)
    e = np.exp(scores)
    w8 = e / e.sum(axis=-1, keepdims=True)
    ctx = np.einsum("bhqk,bhkd->bhqd", w8, v)
    ctx = ctx.transpose(0, 2, 1, 3).reshape(bs, qlen, dim)
    return ctx @ np.asarray(wo, np.float32).T + np.asarray(bo, np.float32)


def run_on_device(inputs, trace=False, **trace_kwargs):
    """Returns (BassKernelResults, combined_output)."""
    nc = _get_program()
    in_maps = _make_in_maps(
        inputs["input"], inputs["wq"], inputs["bq"], inputs["wk"],
        inputs["bk"], inputs["wv"], inputs["bv"], inputs["wo"], inputs["bo"])
    res = bass_utils.run_bass_kernel_spmd(
        nc, in_maps, core_ids=list(range(8)), trace=trace, **trace_kwargs)
    out = _combine(res.results, inputs["bo"])
    return res, out


def kernel(**inputs) -> np.ndarray:
    mask = np.asarray(inputs["mask"])
    if not np.all(mask != 0):
        # fully general (masked) path; the shipped workload always has an
        # all-ones mask so this never triggers on-device sharding
        return _numpy_fallback(**inputs).astype(np.float32)
    _, out = run_on_device(inputs)
    return out


if __name__ == "__main__":
    rng = np.random.default_rng(0)
    ins = {
        "input": rng.normal(size=(BS, S, D)).astype(np.float32),
        "mask": np.ones((BS, S), np.int32),
        "wq": (rng.normal(size=(D, D)) * 0.02).astype(np.float32),
        "bq": (rng.normal(size=(D,)) * 0.02).astype(np.float32),
        "wk": (rng.normal(size=(D, D)) * 0.02).astype(np.float32),
        "bk": (rng.normal(size=(D,)) * 0.02).astype(np.float32),
        "wv": (rng.normal(size=(D, D)) * 0.02).astype(np.float32),
        "bv": (rng.normal(size=(D,)) * 0.02).astype(np.float32),
        "wo": (rng.normal(size=(D, D)) * 0.02).astype(np.float32),
        "bo": (rng.normal(size=(D,)) * 0.02).astype(np.float32),
    }
    out = kernel(**ins)
    exp = _numpy_fallback(**ins)
    err = np.abs(out - exp).max() / np.abs(exp).max()
    print("smoke rel err:", err)
